# revision 13
# baseline (speedup 1.0000x reference)
"""mLSTM block kernel for Trainium2, 8 NeuronCores.

Sharding: batch (4) x head-halves (2) -> 8 cores. Each core handles one
batch element and 8 of the 16 heads: RMSNorm + qkv/gate projections +
chunked mLSTM recurrence (chunk=128) + its half of the output projection.
Host adds the two per-batch partial output projections + residual.

Math (per head, chunk of C=128 positions, inclusive cumsums, all
exponent arguments <= 0 so exp never overflows):
  nlogf = softplus(-softcap(gf)),  NL_t = cumsum(nlogf)
  nlogi, nlogo likewise;  Z_t = NL_t + nlogo_t   (folds o_t = exp(-nlogo))
  r_s = NL_s - nlogi_s
  A'[s,t] = (k_s . q_t) * exp(r_s - Z_t)  for t >= s else 0
  h_t = sum_s A'[s,t] v_s + (q_t * exp(-Z_t)) . S_chunk_start
  S <- exp(-NL_C) * S + sum_s exp(NL_s - NL_C - nlogi_s) k_s v_s^T

RMSNorm trick: xn = inv[s] * x[:, s], so projections run on raw bf16 x
and are scaled by inv at PSUM-evacuation; w_rms is folded into weights
on the host. Gate projections use bf16 hi/lo splitting (3 matmul
groups) because decay errors random-walk over the 128-step cumsum.
"""

import sys

sys.path.insert(0, "/opt/trn_rl_repo")

import numpy as np
import ml_dtypes

import concourse.bass as bass
import concourse.mybir as mybir
import concourse.tile as tile
from concourse import bass_utils
import bass_rust

F32 = mybir.dt.float32
BF16 = mybir.dt.bfloat16
FP16 = mybir.dt.float16
AF = mybir.ActivationFunctionType
OP = mybir.AluOpType

B, S, D = 4, 2048, 1024
H, DH = 16, 64
HL = 8              # heads per core
C = 128             # chunk length
NCHUNK = S // C     # 16
KO = D // 128       # 8 k-tiles over D
FL = HL * DH        # 512 local feature width
EPS = 1e-6
CAP = 15.0
NEG = -30000.0

# ---------------------------------------------------------------------------
# walrus workaround: this compiler build only accepts 1 sync-wait on
# CTRL-lowered instructions (Drain/EventSemaphore/Nop) and rejects >2
# elsewhere. Move the excess onto EventSemaphore carriers inserted before.
# ---------------------------------------------------------------------------
_CTRL_INSTS = ("InstDrain", "InstEventSemaphore", "InstNop")


def _split_excess_waits(nc, max_waits=2):
    n_new = 0
    for fn in nc.m.functions:
        for bb in fn.blocks:
            lst = bb.instructions
            i = 0
            while i < len(lst):
                inst = lst[i]
                si = inst.sync_info
                cap = 1 if type(inst).__name__ in _CTRL_INSTS else max_waits
                if si is None or len(si.on_wait) <= cap:
                    i += 1
                    continue
                waits = list(si.on_wait)
                keep, excess = waits[:cap], waits[cap:]
                carriers = []
                for j in range(len(excess)):
                    ev = mybir.InstEventSemaphore(
                        name=f"I-waitfix-{n_new}", ins=[], outs=[])
                    n_new += 1
                    ev.engine = inst.engine
                    ev.sync_info = bass_rust.SyncInfo(
                        on_wait=[excess[j]], on_update=[])
                    nc.register_instruction(ev, overwrite=True)
                    carriers.append(ev)
                si.on_wait = keep
                for k, ev in enumerate(carriers):
                    lst.insert(i + k, ev)
                i += len(carriers) + 1
    return n_new


# ---------------------------------------------------------------------------
# kernel builder
# ---------------------------------------------------------------------------

def build_nc():
    nc = bass.Bass(target_bir_lowering=False, trn_type="TRN2")

    # wall = [wqk(1024) | wv(512) | wgh(24)] along columns
    WALL = 2 * FL + FL + 3 * HL
    xhi_d = nc.dram_tensor("xhi", [D, S], FP16, kind="ExternalInput")
    wall_d = nc.dram_tensor("wall", [D, WALL], FP16, kind="ExternalInput")
    wo_d = nc.dram_tensor("wo", [FL, D], FP16, kind="ExternalInput")
    s0_d = nc.dram_tensor("s0", [HL, DH, DH], FP16, kind="ExternalInput")
    # cf32 = [triu | e127 | maskneg | idf32], cf16 = [idbf | onesb]
    cf32_d = nc.dram_tensor("cf32", [128, 4, 128], F32, kind="ExternalInput")
    cf16_d = nc.dram_tensor("cf16", [128, 2, 128], FP16, kind="ExternalInput")
    out_d = nc.dram_tensor("outT", [D, S], FP16, kind="ExternalOutput")

    xhi_r = xhi_d.rearrange("(ko p) s -> p ko s", p=128)

    with tile.TileContext(nc) as tc:
        with (
            tc.tile_pool(name="wk", bufs=3) as wk,
            tc.tile_pool(name="dram", bufs=1, space="DRAM") as dpool,
            tc.tile_pool(name="ps_big", bufs=3, space="PSUM") as ps_big,
            tc.tile_pool(name="ps_kt", bufs=2, space="PSUM") as ps_kt,
            tc.tile_pool(name="ps_st", bufs=2, space="PSUM") as ps_st,
        ):
            # ---- persistent tiles --------------------------------------
            frees = []

            def ptile(shape, dtype, name):
                t, f = tc.tile(shape, dtype, name=name)
                frees.append(f)
                return t

            cf32 = ptile([128, 4, 128], F32, "cf32")
            triu, e127, maskneg, idf = (cf32[:, i, :] for i in range(4))
            cf16 = ptile([128, 2, 128], FP16, "cf16")
            idb, onesb = cf16[:, 0, :], cf16[:, 1, :]
            wall = ptile([128, KO, WALL], FP16, "wall")
            wqk = wall[:, :, 0:2 * FL]
            wv = wall[:, :, 2 * FL:3 * FL]
            wgh = wall[:, :, 3 * FL:3 * FL + 3 * HL]
            wo = ptile([128, FL // 128, D], FP16, "wo")
            s_all = ptile([128, 2, HL // 2, DH], FP16, "s_all")
            inv_rep = ptile([128, S], F32, "inv_rep")
            inv_cols = ptile([128, NCHUNK], F32, "inv_cols")
            graw = ptile([24, S], F32, "graw")
            g_cols = ptile([128, NCHUNK, 24], F32, "g_cols")
            gsp = ptile([128, NCHUNK, 24], F32, "gsp")
            nl_cols = ptile([128, 128], F32, "nl_cols")
            r_cols = ptile([128, 128], F32, "r_cols")
            z_cols = ptile([128, 128], F32, "z_cols")
            w_cols = ptile([128, 128], F32, "w_cols")
            exp_nlc = ptile([128, 128], F32, "exp_nlc")
            qkT = ptile([128, KO, S], FP16, "qkT")
            vpos = ptile([128, NCHUNK, FL], FP16, "vpos")
            eps_col = ptile([128, 1], F32, "eps_col")
            nc.vector.memset(eps_col[:], EPS)
            xhi, xhi_free = tc.tile([128, KO, S], FP16, name="xhi")

            # ---- loads -------------------------------------------------
            nc.sync.dma_start(cf32[:], cf32_d[:])
            nc.sync.dma_start(cf16[:], cf16_d[:])
            nc.sync.dma_start(wall[:], wall_d.rearrange("(ko p) j -> p ko j", p=128))
            nc.sync.dma_start(wo[:], wo_d.rearrange("(ko p) j -> p ko j", p=128))
            nc.sync.dma_start(xhi[:, 0:4, :], xhi_r[:, 0:4, :])
            nc.sync.dma_start(xhi[:, 4:8, :], xhi_r[:, 4:8, :])
            # state init: head h -> partitions (h%2)*64+d, index h//2
            nc.sync.dma_start(
                s_all[:, 0, :, :],
                s0_d.rearrange("(a b) d e -> (b d) a e", b=2))

            # ---- variance + inv std ------------------------------------
            for sb in range(4):
                pv = ps_big.tile([128, 512], F32, name="pv", tag="big")
                for k in range(KO):
                    xsq = wk.tile([128, 512], FP16, name="xsq", tag="xsq")
                    nc.scalar.activation(
                        xsq[:], xhi[:, k, sb * 512:(sb + 1) * 512], AF.Square)
                    nc.tensor.matmul(pv[:], onesb, xsq[:],
                                     start=(k == 0), stop=(k == KO - 1))
                lnv = wk.tile([128, 512], F32, name="lnv", tag="lnv")
                nc.scalar.activation(lnv[:], pv[:], AF.Ln,
                                     bias=eps_col[:], scale=1.0 / D)
                nc.scalar.activation(inv_rep[:, sb * 512:(sb + 1) * 512],
                                     lnv[:], AF.Exp, scale=-0.5)
            # inv as per-position columns: [128 s, st]
            for st in range(NCHUNK):
                pt = ps_kt.tile([128, 128], F32, name="pt", tag="kt")
                nc.tensor.transpose(
                    pt[:, 0:1], inv_rep[0:1, st * 128:(st + 1) * 128],
                    cf32[0:1, 3, 0:1])
                nc.vector.tensor_copy(inv_cols[:, st:st + 1], pt[:, 0:1])

            # ---- gate projections (row-major, hi/lo compensated) -------
            for sb in range(4):
                pg = ps_big.tile([24, 512], F32, name="pg", tag="big")
                for k in range(KO):
                    nc.tensor.matmul(pg[:], wgh[:, k, :],
                                     xhi[:, k, sb * 512:(sb + 1) * 512],
                                     start=(k == 0), stop=(k == KO - 1))
                nc.vector.tensor_tensor(
                    graw[:, sb * 512:(sb + 1) * 512], pg[:],
                    inv_rep[0:24, sb * 512:(sb + 1) * 512], OP.mult)
            # transpose to pos-major [128 s, st, 24]
            for st in range(NCHUNK):
                pt = ps_kt.tile([128, 128], F32, name="pt", tag="kt")
                nc.tensor.transpose(
                    pt[:, 0:24], graw[:, st * 128:(st + 1) * 128],
                    cf32[0:24, 3, 0:24])
                nc.scalar.copy(g_cols[:, st, :], pt[:, 0:24])
            # softcap -> log-gates: gsp = ln(sigmoid(15*tanh(g/15))) <= 0
            # (CoreSim lacks Softplus; -gsp is the neg-log gate)
            t1 = wk.tile([128, NCHUNK, 24], F32, name="t1", tag="t1")
            nc.scalar.activation(t1[:], g_cols[:], AF.Tanh, scale=1.0 / CAP)
            sg = wk.tile([128, NCHUNK, 24], F32, name="sg", tag="t1")
            nc.scalar.activation(sg[:], t1[:], AF.Sigmoid, scale=CAP)
            nc.scalar.activation(gsp[:], sg[:], AF.Ln)
            lns_i = gsp[:, :, 0:HL]
            lns_f = gsp[:, :, HL:2 * HL]
            lns_o = gsp[:, :, 2 * HL:3 * HL]

            # ---- cumsums / decay tables --------------------------------
            # triu holds -1 on s<=t, so NL = -cumsum(ln f) >= 0
            pnl = ps_big.tile([128, 512], F32, name="pnl", tag="big")
            nc.tensor.matmul(pnl[:, 0:128], triu, lns_f,
                             start=True, stop=True)
            nc.vector.tensor_copy(nl_cols[:], pnl[:, 0:128])
            nc.vector.tensor_tensor(r_cols[:], nl_cols[:], lns_i, OP.add)
            nc.vector.tensor_tensor(z_cols[:], nl_cols[:], lns_o, OP.subtract)
            pnlc = ps_big.tile([128, 512], F32, name="pnlc", tag="big")
            nc.tensor.matmul(pnlc[:, 0:128], e127, nl_cols[:],
                             start=True, stop=True)
            w_tmp = wk.tile([128, 128], F32, name="w_tmp", tag="tmp")
            nc.vector.tensor_tensor(w_tmp[:], r_cols[:], pnlc[:, 0:128],
                                    OP.subtract)
            nc.scalar.activation(w_cols[:], w_tmp[:], AF.Exp)
            nc.scalar.activation(exp_nlc[:], pnlc[:, 0:128], AF.Exp,
                                 scale=-1.0)
            # Z rows -> DRAM for partition-broadcast loads
            pzr = ps_kt.tile([128, 128], F32, name="pzr", tag="kt")
            nc.tensor.transpose(pzr[:], z_cols[:], idf)
            zr_sb = wk.tile([128, 128], F32, name="zr_sb", tag="tmp")
            nc.vector.tensor_copy(zr_sb[:], pzr[:])
            zr_dram = dpool.tile([128, 128], F32, name="zr_dram")
            nc.sync.dma_start(zr_dram[:], zr_sb[:])

            # ---- q|k projection (feature-major) ------------------------
            for jt in range(KO):
                for sb in range(4):
                    pq = ps_big.tile([128, 512], F32, name="pq", tag="big")
                    for k in range(KO):
                        nc.tensor.matmul(
                            pq[:], wqk[:, k, jt * 128:(jt + 1) * 128],
                            xhi[:, k, sb * 512:(sb + 1) * 512],
                            start=(k == 0), stop=(k == KO - 1))
                    nc.vector.tensor_tensor(
                        qkT[:, jt, sb * 512:(sb + 1) * 512], pq[:],
                        inv_rep[:, sb * 512:(sb + 1) * 512], OP.mult)

            # ---- v projection (position-major) -------------------------
            for st in range(NCHUNK):
                pvp = ps_big.tile([128, 512], F32, name="pvp", tag="big")
                for k in range(KO):
                    nc.tensor.matmul(
                        pvp[:], xhi[:, k, st * 128:(st + 1) * 128],
                        wv[:, k, :],
                        start=(k == 0), stop=(k == KO - 1))
                nc.vector.tensor_scalar_mul(
                    vpos[:, st, :], pvp[:], inv_cols[:, st:st + 1])

            xhi_free()
            hT, hT_free = tc.tile([128, FL // 128, S], FP16, name="hT")

            # ---- recurrence over chunks --------------------------------
            for c in range(NCHUNK):
                if c % 2 == 0:
                    repz = wk.tile([128, 2 * HL, 128], F32, name="repz",
                                   tag="repz", bufs=2)
                    nc.sync.dma_start(
                        repz[:],
                        zr_dram[None, c * HL:(c + 2) * HL, :]
                        .to_broadcast((128, 2 * HL, 128)))
                    repez = wk.tile([128, 2 * HL, 128], F32, name="repez",
                                    tag="repez", bufs=2)
                    nc.scalar.activation(repez[:], repz[:], AF.Exp,
                                         scale=-1.0)
                for h in range(HL):
                    p = c * HL + h
                    hh = (c % 2) * HL + h
                    po = (h % 2) * 64
                    qs = qkT[po:po + 64, h // 2, c * 128:(c + 1) * 128]
                    ks = qkT[po:po + 64, 4 + h // 2, c * 128:(c + 1) * 128]
                    vs = vpos[:, c, h * DH:(h + 1) * DH]
                    s_old = s_all[po:po + 64, c % 2, h // 2, :]
                    s_new = s_all[po:po + 64, (c + 1) % 2, h // 2, :]

                    # attention scores (k.q) and decay matrix
                    pa = ps_big.tile([128, 512], F32, name="pa", tag="big")
                    nc.tensor.matmul(pa[:, 0:128], ks, qs,
                                     start=True, stop=True)
                    tmp = wk.tile([128, 128], F32, name="tmp", tag="tmp")
                    nc.vector.tensor_tensor(tmp[:], maskneg,
                                            repz[:, hh, :], OP.subtract)
                    expd = wk.tile([128, 128], F32, name="expd", tag="expd")
                    nc.scalar.activation(expd[:], tmp[:], AF.Exp,
                                         bias=r_cols[:, p:p + 1])
                    a_bf = wk.tile([128, 128], FP16, name="a_bf", tag="a_bf")
                    nc.vector.tensor_tensor(a_bf[:], pa[:, 0:128], expd[:],
                                            OP.mult)

                    # hT[e,t] = V^T A' + S^T (q*exp(-Z))  (scales pre-folded)
                    qt_bf = wk.tile([128, 128], FP16, name="qt_bf",
                                    tag="qt_bf")
                    nc.vector.tensor_tensor(qt_bf[po:po + 64, :], qs,
                                            repez[po:po + 64, hh, :], OP.mult)
                    pht = ps_kt.tile([128, 128], F32, name="pht", tag="kt")
                    nc.tensor.matmul(pht[po:po + 64, 0:128], vs, a_bf[:],
                                     start=True, stop=False,
                                     tile_position=(0, po))
                    nc.tensor.matmul(pht[po:po + 64, 0:128], s_old,
                                     qt_bf[po:po + 64, :],
                                     start=False, stop=True,
                                     tile_position=(po, po))
                    nc.scalar.copy(
                        hT[po:po + 64, h // 2, c * 128:(c + 1) * 128],
                        pht[po:po + 64, 0:128])

                    # state update
                    pkp = ps_kt.tile([128, 128], FP16, name="pkp", tag="kt")
                    nc.tensor.transpose(pkp[:, 0:DH], ks,
                                        idb[po:po + 64, po:po + 64])
                    kt_bf = wk.tile([128, DH], FP16, name="kt_bf",
                                    tag="kt_bf")
                    nc.vector.tensor_scalar_mul(kt_bf[:], pkp[:, 0:DH],
                                                w_cols[:, p:p + 1])
                    pst = ps_st.tile([128, DH], F32, name="pst", tag="st")
                    nc.tensor.matmul(pst[po:po + 64, :], kt_bf[:], vs,
                                     start=True, stop=True,
                                     tile_position=(0, po))
                    nc.vector.scalar_tensor_tensor(
                        s_new, s_old, exp_nlc[po:po + 64, p:p + 1],
                        pst[po:po + 64, :], OP.mult, OP.add)

            # ---- output projection -------------------------------------
            for jt in range(KO):
                osb = wk.tile([128, S], FP16, name="osb", tag="osb", bufs=2)
                for sb in range(4):
                    pout = ps_big.tile([128, 512], F32, name="pout",
                                       tag="big")
                    for kk in range(FL // 128):
                        nc.tensor.matmul(
                            pout[:], wo[:, kk, jt * 128:(jt + 1) * 128],
                            hT[:, kk, sb * 512:(sb + 1) * 512],
                            start=(kk == 0), stop=(kk == FL // 128 - 1))
                    nc.vector.tensor_copy(
                        osb[:, sb * 512:(sb + 1) * 512], pout[:])
                nc.sync.dma_start(out_d[jt * 128:(jt + 1) * 128, :], osb[:])

            hT_free()
            for f in reversed(frees):
                f()

    _split_excess_waits(nc)
    nc.finalize()
    return nc


# ---------------------------------------------------------------------------
# host-side constants and shard prep
# ---------------------------------------------------------------------------

def _consts():
    i = np.arange(128)
    triu = -(i[:, None] <= i[None, :]).astype(np.float32)      # [s,t] s<=t
    e127 = np.zeros((128, 128), np.float32)
    e127[127, :] = 1.0
    maskneg = np.where(i[:, None] <= i[None, :], 0.0, NEG).astype(np.float32)
    idf = np.eye(128, dtype=np.float32)
    cf32 = np.stack([triu, e127, maskneg, idf], axis=1)
    cf16 = np.stack([np.eye(128, dtype=np.float16),
                     np.ones((128, 128), np.float16)], axis=1)
    return dict(cf32=np.ascontiguousarray(cf32),
                cf16=np.ascontiguousarray(cf16))


def _bf(x):
    return np.asarray(x, dtype=np.float16)


_NC_CACHE = None


def kernel(x, hidden_state, w_rms, w_qkv, w_gate, w_out):
    global _NC_CACHE
    x = np.asarray(x, np.float32)
    hidden_state = np.asarray(hidden_state, np.float32)
    w_rms = np.asarray(w_rms, np.float32)
    w_qkv = np.asarray(w_qkv, np.float32)
    w_gate = np.asarray(w_gate, np.float32)
    w_out = np.asarray(w_out, np.float32)

    if _NC_CACHE is None:
        _NC_CACHE = build_nc()
    nc = _NC_CACHE

    consts = _consts()
    wq3 = (w_rms[:, None] * w_qkv).reshape(D, 3, H, DH)
    wg3 = (w_rms[:, None] * w_gate).reshape(D, 3, H)

    in_maps = []
    for core in range(8):
        b, hg = core // 2, core % 2
        h0 = hg * HL
        xT = np.ascontiguousarray(x[b].T)                      # [D, S]
        wall = np.concatenate(
            [wq3[:, 0, h0:h0 + HL, :].reshape(D, FL),
             wq3[:, 1, h0:h0 + HL, :].reshape(D, FL),
             wq3[:, 2, h0:h0 + HL, :].reshape(D, FL),
             wg3[:, :, h0:h0 + HL].reshape(D, 3 * HL)], axis=1)  # [i8|f8|o8]
        m = dict(
            xhi=_bf(xT), wall=_bf(wall),
            wo=_bf(w_out[h0 * DH:(h0 + HL) * DH, :]),
            s0=_bf(hidden_state[b, h0:h0 + HL]), **consts)
        in_maps.append(m)

    res = bass_utils.run_bass_kernel_spmd(nc, in_maps, core_ids=list(range(8)))

    out = np.empty((B, S, D), np.float32)
    for b in range(B):
        acc = (res.results[2 * b]["outT"].astype(np.float32)
               + res.results[2 * b + 1]["outT"].astype(np.float32))
        out[b] = x[b] + acc.T
    return out


# revision 17
# speedup vs baseline: 1.1113x; 1.1113x over previous
"""mLSTM block kernel for Trainium2, 8 NeuronCores.

Sharding: batch (4) x head-halves (2) -> 8 cores. Each core handles one
batch element and 8 of the 16 heads: RMSNorm + qkv/gate projections +
chunked mLSTM recurrence (chunk=128) + its half of the output projection.
Host adds the two per-batch partial output projections + residual.

Math (per head, chunk of C=128 positions, inclusive cumsums, all
exponent arguments <= 0 so exp never overflows):
  nlogf = softplus(-softcap(gf)),  NL_t = cumsum(nlogf)
  nlogi, nlogo likewise;  Z_t = NL_t + nlogo_t   (folds o_t = exp(-nlogo))
  r_s = NL_s - nlogi_s
  A'[s,t] = (k_s . q_t) * exp(r_s - Z_t)  for t >= s else 0
  h_t = sum_s A'[s,t] v_s + (q_t * exp(-Z_t)) . S_chunk_start
  S <- exp(-NL_C) * S + sum_s exp(NL_s - NL_C - nlogi_s) k_s v_s^T

RMSNorm trick: xn = inv[s] * x[:, s], so projections run on raw bf16 x
and are scaled by inv at PSUM-evacuation; w_rms is folded into weights
on the host. Gate projections use bf16 hi/lo splitting (3 matmul
groups) because decay errors random-walk over the 128-step cumsum.
"""

import sys

sys.path.insert(0, "/opt/trn_rl_repo")

import numpy as np
import ml_dtypes

import concourse.bass as bass
import concourse.mybir as mybir
import concourse.tile as tile
from concourse import bass_utils
import bass_rust

F32 = mybir.dt.float32
BF16 = mybir.dt.bfloat16
FP16 = mybir.dt.float16
AF = mybir.ActivationFunctionType
OP = mybir.AluOpType

B, S, D = 4, 2048, 1024
H, DH = 16, 64
HL = 8              # heads per core
C = 128             # chunk length
NCHUNK = S // C     # 16
KO = D // 128       # 8 k-tiles over D
FL = HL * DH        # 512 local feature width
EPS = 1e-6
CAP = 15.0
NEG = -30000.0

# ---------------------------------------------------------------------------
# walrus workaround: this compiler build only accepts 1 sync-wait on
# CTRL-lowered instructions (Drain/EventSemaphore/Nop) and rejects >2
# elsewhere. Move the excess onto EventSemaphore carriers inserted before.
# ---------------------------------------------------------------------------
_CTRL_INSTS = ("InstDrain", "InstEventSemaphore", "InstNop")


def _split_excess_waits(nc, max_waits=2):
    n_new = 0
    for fn in nc.m.functions:
        for bb in fn.blocks:
            lst = bb.instructions
            i = 0
            while i < len(lst):
                inst = lst[i]
                si = inst.sync_info
                cap = 1 if type(inst).__name__ in _CTRL_INSTS else max_waits
                if si is None or len(si.on_wait) <= cap:
                    i += 1
                    continue
                waits = list(si.on_wait)
                keep, excess = waits[:cap], waits[cap:]
                carriers = []
                for j in range(len(excess)):
                    ev = mybir.InstEventSemaphore(
                        name=f"I-waitfix-{n_new}", ins=[], outs=[])
                    n_new += 1
                    ev.engine = inst.engine
                    ev.sync_info = bass_rust.SyncInfo(
                        on_wait=[excess[j]], on_update=[])
                    nc.register_instruction(ev, overwrite=True)
                    carriers.append(ev)
                si.on_wait = keep
                for k, ev in enumerate(carriers):
                    lst.insert(i + k, ev)
                i += len(carriers) + 1
    return n_new


# ---------------------------------------------------------------------------
# kernel builder
# ---------------------------------------------------------------------------

def build_nc():
    nc = bass.Bass(target_bir_lowering=False, trn_type="TRN2")

    # wall = [wqk(1024) | wv(512) | wgh(24)] along columns
    WALL = 2 * FL + FL + 3 * HL
    xhi_d = nc.dram_tensor("xhi", [D, S], FP16, kind="ExternalInput")
    wall_d = nc.dram_tensor("wall", [D, WALL], FP16, kind="ExternalInput")
    wo_d = nc.dram_tensor("wo", [FL, D], FP16, kind="ExternalInput")
    s0_d = nc.dram_tensor("s0", [HL, DH, DH], FP16, kind="ExternalInput")
    # cf32 = [triu | e127 | maskneg | idf32], cf16 = [idbf | onesb]
    cf32_d = nc.dram_tensor("cf32", [128, 4, 128], F32, kind="ExternalInput")
    cf16_d = nc.dram_tensor("cf16", [128, 2, 128], FP16, kind="ExternalInput")
    out_d = nc.dram_tensor("outT", [D, S], FP16, kind="ExternalOutput")

    xhi_r = xhi_d.rearrange("(ko p) s -> p ko s", p=128)

    with tile.TileContext(nc) as tc:
        with (
            tc.tile_pool(name="wk", bufs=3) as wk,
            tc.tile_pool(name="dram", bufs=1, space="DRAM") as dpool,
            tc.tile_pool(name="ps_big", bufs=2, space="PSUM") as ps_big,
            tc.tile_pool(name="ps_kt", bufs=2, space="PSUM") as ps_kt,
            tc.tile_pool(name="ps_st", bufs=2, space="PSUM") as ps_st,
        ):
            # ---- persistent tiles --------------------------------------
            frees = []

            def ptile(shape, dtype, name):
                t, f = tc.tile(shape, dtype, name=name)
                frees.append(f)
                return t

            cf32 = ptile([128, 4, 128], F32, "cf32")
            triu, e127, maskneg, idf = (cf32[:, i, :] for i in range(4))
            cf16 = ptile([128, 2, 128], FP16, "cf16")
            idb, onesb = cf16[:, 0, :], cf16[:, 1, :]
            wall = ptile([128, KO, WALL], FP16, "wall")
            wqk = wall[:, :, 0:2 * FL]
            wv = wall[:, :, 2 * FL:3 * FL]
            wgh = wall[:, :, 3 * FL:3 * FL + 3 * HL]
            wo = ptile([128, FL // 128, D], FP16, "wo")
            s_all = ptile([128, 2, HL // 2, DH], FP16, "s_all")
            inv_rep = ptile([128, S], F32, "inv_rep")
            inv_cols = ptile([128, NCHUNK], F32, "inv_cols")
            graw = ptile([24, S], F32, "graw")
            g_cols = ptile([128, NCHUNK, 24], F32, "g_cols")
            gsp = ptile([128, NCHUNK, 24], F32, "gsp")
            nl_cols = ptile([128, 128], F32, "nl_cols")
            r_cols = ptile([128, 128], F32, "r_cols")
            z_cols = ptile([128, 128], F32, "z_cols")
            w_cols = ptile([128, 128], F32, "w_cols")
            exp_nlc = ptile([128, 128], F32, "exp_nlc")
            qkT = ptile([128, KO, S], FP16, "qkT")
            vpos = ptile([128, NCHUNK, FL], FP16, "vpos")
            eps_col = ptile([128, 1], F32, "eps_col")
            nc.vector.memset(eps_col[:], EPS)
            xhi, xhi_free = tc.tile([128, KO, S], FP16, name="xhi")

            # ---- loads -------------------------------------------------
            nc.sync.dma_start(cf32[:], cf32_d[:])
            nc.sync.dma_start(cf16[:], cf16_d[:])
            nc.sync.dma_start(wall[:], wall_d.rearrange("(ko p) j -> p ko j", p=128))
            nc.sync.dma_start(wo[:], wo_d.rearrange("(ko p) j -> p ko j", p=128))
            nc.sync.dma_start(xhi[:, 0:4, :], xhi_r[:, 0:4, :])
            nc.sync.dma_start(xhi[:, 4:8, :], xhi_r[:, 4:8, :])
            # state init: head h -> partitions (h%2)*64+d, index h//2
            nc.sync.dma_start(
                s_all[:, 0, :, :],
                s0_d.rearrange("(a b) d e -> (b d) a e", b=2))

            # ---- variance + inv std ------------------------------------
            for sb in range(4):
                pv = ps_big.tile([128, 512], F32, name="pv", tag="big")
                for k in range(KO):
                    xsq = wk.tile([128, 512], FP16, name="xsq", tag="xsq")
                    nc.scalar.activation(
                        xsq[:], xhi[:, k, sb * 512:(sb + 1) * 512], AF.Square)
                    nc.tensor.matmul(pv[:], onesb, xsq[:],
                                     start=(k == 0), stop=(k == KO - 1))
                lnv = wk.tile([128, 512], F32, name="lnv", tag="lnv",
                              bufs=2)
                nc.scalar.activation(lnv[:], pv[:], AF.Ln,
                                     bias=eps_col[:], scale=1.0 / D)
                nc.scalar.activation(inv_rep[:, sb * 512:(sb + 1) * 512],
                                     lnv[:], AF.Exp, scale=-0.5)
            # inv as per-position columns: [128 s, st]
            for st in range(NCHUNK):
                pt = ps_kt.tile([128, 128], F32, name="pt", tag="kt")
                nc.tensor.transpose(
                    pt[:, 0:1], inv_rep[0:1, st * 128:(st + 1) * 128],
                    cf32[0:1, 3, 0:1])
                nc.vector.tensor_copy(inv_cols[:, st:st + 1], pt[:, 0:1])

            # ---- gate projections (row-major, hi/lo compensated) -------
            for sb in range(4):
                pg = ps_big.tile([24, 512], F32, name="pg", tag="big")
                for k in range(KO):
                    nc.tensor.matmul(pg[:], wgh[:, k, :],
                                     xhi[:, k, sb * 512:(sb + 1) * 512],
                                     start=(k == 0), stop=(k == KO - 1))
                nc.vector.tensor_tensor(
                    graw[:, sb * 512:(sb + 1) * 512], pg[:],
                    inv_rep[0:24, sb * 512:(sb + 1) * 512], OP.mult)
            # transpose to pos-major [128 s, st, 24]
            for st in range(NCHUNK):
                pt = ps_kt.tile([128, 128], F32, name="pt", tag="kt")
                nc.tensor.transpose(
                    pt[:, 0:24], graw[:, st * 128:(st + 1) * 128],
                    cf32[0:24, 3, 0:24])
                nc.scalar.copy(g_cols[:, st, :], pt[:, 0:24])
            # softcap -> log-gates: gsp = ln(sigmoid(15*tanh(g/15))) <= 0
            # (CoreSim lacks Softplus; -gsp is the neg-log gate)
            t1 = wk.tile([128, NCHUNK, 24], F32, name="t1", tag="t1",
                         bufs=1)
            nc.scalar.activation(t1[:], g_cols[:], AF.Tanh, scale=1.0 / CAP)
            sg = wk.tile([128, NCHUNK, 24], F32, name="sg", tag="sg",
                         bufs=1)
            nc.scalar.activation(sg[:], t1[:], AF.Sigmoid, scale=CAP)
            nc.scalar.activation(gsp[:], sg[:], AF.Ln)
            lns_i = gsp[:, :, 0:HL]
            lns_f = gsp[:, :, HL:2 * HL]
            lns_o = gsp[:, :, 2 * HL:3 * HL]

            # ---- cumsums / decay tables --------------------------------
            # triu holds -1 on s<=t, so NL = -cumsum(ln f) >= 0
            pnl = ps_big.tile([128, 512], F32, name="pnl", tag="big")
            nc.tensor.matmul(pnl[:, 0:128], triu, lns_f,
                             start=True, stop=True)
            nc.vector.tensor_copy(nl_cols[:], pnl[:, 0:128])
            nc.vector.tensor_tensor(r_cols[:], nl_cols[:], lns_i, OP.add)
            nc.vector.tensor_tensor(z_cols[:], nl_cols[:], lns_o, OP.subtract)
            pnlc = ps_big.tile([128, 512], F32, name="pnlc", tag="big")
            nc.tensor.matmul(pnlc[:, 0:128], e127, nl_cols[:],
                             start=True, stop=True)
            w_tmp = wk.tile([128, 128], F32, name="w_tmp", tag="tmp")
            nc.vector.tensor_tensor(w_tmp[:], r_cols[:], pnlc[:, 0:128],
                                    OP.subtract)
            nc.scalar.activation(w_cols[:], w_tmp[:], AF.Exp)
            nc.scalar.activation(exp_nlc[:], pnlc[:, 0:128], AF.Exp,
                                 scale=-1.0)
            # Z rows -> DRAM for partition-broadcast loads
            pzr = ps_kt.tile([128, 128], F32, name="pzr", tag="kt")
            nc.tensor.transpose(pzr[:], z_cols[:], idf)
            zr_sb = wk.tile([128, 128], F32, name="zr_sb", tag="tmp")
            nc.vector.tensor_copy(zr_sb[:], pzr[:])
            zr_dram = dpool.tile([128, 128], F32, name="zr_dram")
            nc.sync.dma_start(zr_dram[:], zr_sb[:])

            # ---- q|k projection (feature-major) ------------------------
            for jt in range(KO):
                for sb in range(4):
                    pq = ps_big.tile([128, 512], F32, name="pq", tag="big")
                    for k in range(KO):
                        nc.tensor.matmul(
                            pq[:], wqk[:, k, jt * 128:(jt + 1) * 128],
                            xhi[:, k, sb * 512:(sb + 1) * 512],
                            start=(k == 0), stop=(k == KO - 1))
                    nc.vector.tensor_tensor(
                        qkT[:, jt, sb * 512:(sb + 1) * 512], pq[:],
                        inv_rep[:, sb * 512:(sb + 1) * 512], OP.mult)

            # ---- v projection (position-major) -------------------------
            for st in range(NCHUNK):
                pvp = ps_big.tile([128, 512], F32, name="pvp", tag="big")
                for k in range(KO):
                    nc.tensor.matmul(
                        pvp[:], xhi[:, k, st * 128:(st + 1) * 128],
                        wv[:, k, :],
                        start=(k == 0), stop=(k == KO - 1))
                nc.vector.tensor_scalar_mul(
                    vpos[:, st, :], pvp[:], inv_cols[:, st:st + 1])

            xhi_free()
            hT, hT_free = tc.tile([128, FL // 128, S], FP16, name="hT")

            # ---- recurrence over chunks --------------------------------
            # waves of 4 heads sharing a partition window: h = 2k + h0
            r_v = r_cols.rearrange("s (c k two) -> s c k two", k=4, two=2)
            w_v = w_cols.rearrange("s (c k two) -> s c k two", k=4, two=2)
            e_v = exp_nlc.rearrange("s (c k two) -> s c k two", k=4, two=2)
            for c in range(NCHUNK):
                if c % 2 == 0:
                    repz = wk.tile([128, 2, 4, 2, 128], F32, name="repz",
                                   tag="repz", bufs=2)
                    nc.sync.dma_start(
                        repz[:],
                        zr_dram[None, c * HL:(c + 2) * HL, :]
                        .to_broadcast((128, 2 * HL, 128))
                        .rearrange("s (a k two) t -> s a k two t", k=4, two=2))
                    repez = wk.tile([128, 2, 4, 2, 128], FP16, name="repez",
                                    tag="repez", bufs=2)
                    nc.scalar.activation(repez[:], repz[:], AF.Exp,
                                         scale=-1.0)
                cs = slice(c * 128, (c + 1) * 128)
                for h0 in range(2):
                    po = h0 * 64
                    pw = slice(po, po + 64)
                    qs_w = qkT[pw, 0:4, cs]
                    ks_w = qkT[pw, 4:8, cs]
                    rz_w = repz[:, c % 2, :, h0, :]
                    rez_w = repez[pw, c % 2, :, h0, :]
                    r_bc = r_v[:, c, :, h0, None].to_broadcast((128, 4, 128))
                    wc_bc = w_v[:, c, :, h0, None].to_broadcast((128, 4, 64))
                    en_bc = e_v[pw, c, :, h0, None].to_broadcast((64, 4, 64))
                    s_old_w = s_all[pw, c % 2, :, :]
                    s_new_w = s_all[pw, (c + 1) % 2, :, :]

                    # attention scores (k.q), 4 heads into one bank
                    pa = ps_big.tile([128, 4, 128], F32, name="pa", tag="big")
                    for k in range(4):
                        nc.tensor.matmul(pa[:, k, :], ks_w[:, k, :],
                                         qs_w[:, k, :], start=True, stop=True,
                                         skip_group_check=True)
                    # decay matrix exp(r_s - Z_t + mask), batched
                    d1 = wk.tile([128, 4, 128], F32, name="d1", tag="d1")
                    nc.vector.tensor_tensor(d1[:], r_bc, rz_w, OP.subtract)
                    nc.vector.tensor_tensor(
                        d1[:], d1[:],
                        maskneg[:, None, :].to_broadcast((128, 4, 128)),
                        OP.add)
                    expd = wk.tile([128, 4, 128], FP16, name="expd",
                                   tag="expd")
                    nc.scalar.activation(expd[:], d1[:], AF.Exp)
                    a_w = wk.tile([128, 4, 128], FP16, name="a_w", tag="a_w")
                    nc.vector.tensor_tensor(a_w[:], pa[:], expd[:], OP.mult)

                    # hT[e,t] = V^T A' + S^T (q*exp(-Z))
                    qt_w = wk.tile([128, 4, 128], FP16, name="qt_w",
                                   tag="qt_w")
                    nc.vector.tensor_tensor(qt_w[pw, :, :], qs_w, rez_w,
                                            OP.mult)
                    pht = ps_kt.tile([128, 4, 128], F32, name="pht",
                                     tag="kt")
                    for k in range(4):
                        vs = vpos[:, c, (2 * k + h0) * DH:
                                  (2 * k + h0 + 1) * DH]
                        nc.tensor.matmul(pht[pw, k, :], vs, a_w[:, k, :],
                                         start=True, stop=False,
                                         tile_position=(0, po),
                                         skip_group_check=True)
                        nc.tensor.matmul(pht[pw, k, :], s_old_w[:, k, :],
                                         qt_w[pw, k, :],
                                         start=False, stop=True,
                                         tile_position=(po, po),
                                         skip_group_check=True)
                    nc.scalar.copy(hT[pw, :, cs], pht[pw, :, :])

                    # state update: S <- exp(-NL_C) S + K~^T V
                    pkp = ps_st.tile([128, 4, DH], FP16, name="pkp",
                                     tag="pkp")
                    for k in range(4):
                        nc.tensor.matmul(pkp[:, k, :], ks_w[:, k, :],
                                         idb[pw, pw], is_transpose=True,
                                         skip_group_check=True)
                    ktb = wk.tile([128, 4, DH], FP16, name="ktb", tag="ktb")
                    nc.vector.tensor_tensor(ktb[:], pkp[:], wc_bc, OP.mult)
                    pst = ps_st.tile([128, 4, DH], F32, name="pst",
                                     tag="pst")
                    for k in range(4):
                        vs = vpos[:, c, (2 * k + h0) * DH:
                                  (2 * k + h0 + 1) * DH]
                        nc.tensor.matmul(pst[pw, k, :], ktb[:, k, :], vs,
                                         start=True, stop=True,
                                         tile_position=(0, po),
                                         skip_group_check=True)
                    sdec = wk.tile([128, 4, DH], F32, name="sdec", tag="sdec")
                    nc.vector.tensor_tensor(sdec[pw, :, :], s_old_w, en_bc,
                                            OP.mult)
                    nc.vector.tensor_tensor(s_new_w, sdec[pw, :, :],
                                            pst[pw, :, :], OP.add)

            # ---- output projection -------------------------------------
            for jt in range(KO):
                osb = wk.tile([128, S], FP16, name="osb", tag="osb", bufs=2)
                for sb in range(4):
                    pout = ps_big.tile([128, 512], F32, name="pout",
                                       tag="big")
                    for kk in range(FL // 128):
                        nc.tensor.matmul(
                            pout[:], wo[:, kk, jt * 128:(jt + 1) * 128],
                            hT[:, kk, sb * 512:(sb + 1) * 512],
                            start=(kk == 0), stop=(kk == FL // 128 - 1))
                    nc.vector.tensor_copy(
                        osb[:, sb * 512:(sb + 1) * 512], pout[:])
                nc.sync.dma_start(out_d[jt * 128:(jt + 1) * 128, :], osb[:])

            hT_free()
            for f in reversed(frees):
                f()

    _split_excess_waits(nc)
    nc.finalize()
    return nc


# ---------------------------------------------------------------------------
# host-side constants and shard prep
# ---------------------------------------------------------------------------

def _consts():
    i = np.arange(128)
    triu = -(i[:, None] <= i[None, :]).astype(np.float32)      # [s,t] s<=t
    e127 = np.zeros((128, 128), np.float32)
    e127[127, :] = 1.0
    maskneg = np.where(i[:, None] <= i[None, :], 0.0, NEG).astype(np.float32)
    idf = np.eye(128, dtype=np.float32)
    cf32 = np.stack([triu, e127, maskneg, idf], axis=1)
    cf16 = np.stack([np.eye(128, dtype=np.float16),
                     np.ones((128, 128), np.float16)], axis=1)
    return dict(cf32=np.ascontiguousarray(cf32),
                cf16=np.ascontiguousarray(cf16))


def _bf(x):
    return np.asarray(x, dtype=np.float16)


_NC_CACHE = None


def kernel(x, hidden_state, w_rms, w_qkv, w_gate, w_out):
    global _NC_CACHE
    x = np.asarray(x, np.float32)
    hidden_state = np.asarray(hidden_state, np.float32)
    w_rms = np.asarray(w_rms, np.float32)
    w_qkv = np.asarray(w_qkv, np.float32)
    w_gate = np.asarray(w_gate, np.float32)
    w_out = np.asarray(w_out, np.float32)

    if _NC_CACHE is None:
        _NC_CACHE = build_nc()
    nc = _NC_CACHE

    consts = _consts()
    wq3 = (w_rms[:, None] * w_qkv).reshape(D, 3, H, DH)
    wg3 = (w_rms[:, None] * w_gate).reshape(D, 3, H)

    in_maps = []
    for core in range(8):
        b, hg = core // 2, core % 2
        h0 = hg * HL
        xT = np.ascontiguousarray(x[b].T)                      # [D, S]
        wall = np.concatenate(
            [wq3[:, 0, h0:h0 + HL, :].reshape(D, FL),
             wq3[:, 1, h0:h0 + HL, :].reshape(D, FL),
             wq3[:, 2, h0:h0 + HL, :].reshape(D, FL),
             wg3[:, :, h0:h0 + HL].reshape(D, 3 * HL)], axis=1)  # [i8|f8|o8]
        m = dict(
            xhi=_bf(xT), wall=_bf(wall),
            wo=_bf(w_out[h0 * DH:(h0 + HL) * DH, :]),
            s0=_bf(hidden_state[b, h0:h0 + HL]), **consts)
        in_maps.append(m)

    res = bass_utils.run_bass_kernel_spmd(nc, in_maps, core_ids=list(range(8)))

    out = np.empty((B, S, D), np.float32)
    for b in range(B):
        acc = (res.results[2 * b]["outT"].astype(np.float32)
               + res.results[2 * b + 1]["outT"].astype(np.float32))
        out[b] = x[b] + acc.T
    return out
